# revision 28
# baseline (speedup 1.0000x reference)
"""PCEN kernel for Trainium2, sharded over the time axis across 8 NeuronCores.

Strategy (v2):
  - data is [B=8, F=128, T=16384] fp32. Each core owns a T-slice of 2048 cols
    (all batches). Inputs are downcast to bf16 on the host and outputs are
    shipped back as bf16 (rel tolerance is 2e-2; bf16 rounding is ~2e-3 max),
    halving DMA traffic.
  - The EMA smoother M uses only batch 0. Each core computes M for its own
    slice independently via a 32-col halo: contributions older than 32 steps
    are attenuated by (1-s)^32 ~ 2e-10. The scan is 4 chained
    TensorTensorScan chunks (fp32 state), with x0h DMA'd in 4 matching
    column-chunks so scan chunk c starts as soon as its slice lands
    (per-chunk 1-col probe copies carry the DMA waits; the scan ISA struct
    has no wait slots).
  - R = 1/M^alpha = exp(-alpha*ln(M+eps)) via per-chunk Ln+Exp on ACT. A
    manual InstLoadActFuncSet pins the natural_log_exp_and_others table at
    program start (overlapped with the DMA wait), so Ln->Exp needs NO table
    reload and the chunks pipeline behind the scan; only the Sqrt table
    load (1.28us) remains, right after the last Exp.
  - Chunk widths [672|640|512|256] (first includes the 32-col halo): the
    last chunk is small so the trailing Ln+Exp adds only ~0.8us after the
    scan instead of a full-width 2us Ln + 2us Exp.
  - Per batch b: E = x*R (DVE bf16 2x mode), U = sqrt(E + delta) (ACT fp32),
    out = U - delta^r (DVE tensor_scalar, fp32 in / bf16 out). The subtract
    must read fp32 U: near-zero outputs cancel (U ~ sqrt(delta)), so a bf16
    U would blow up the relative error. GPSIMD is excluded from the hot
    path: its tensor_scalar runs at ~9 G elem/s vs DVE's ~240 G elem/s.
  - Batches 1..6 are processed as pairs in [F, 4096] tiles: one Sqrt and one
    sub per pair halves the per-instruction overhead (ACT pays 222
    init-cycles per instruction). The last batch's sub is split into two
    [F, 1024] halves so the final output DMA is small (shorter drain).
  - All input DMAs are queued upfront on the sync (SP) HWDGE ring, x0h
    chunks first then batches in compute order: the per-queue rings drain
    FIFO, so x0h still lands first without a serializing gate DMA.
"""

import sys

if "/opt/trn_rl_repo" not in sys.path:
    sys.path.insert(0, "/opt/trn_rl_repo")

from contextlib import ExitStack

import ml_dtypes
import numpy as np

import concourse.bass as bass
import concourse.mybir as mybir
import concourse.tile as tile
from concourse import bacc
from concourse.bass_utils import run_bass_kernel_spmd

B, F, T = 8, 128, 16384
NCORES = 8
TLOC = T // NCORES  # 2048 cols per core
HALO = 32  # scan warmup; (1-s)^32 ~ 2e-10 << 2e-2 tolerance
TH = TLOC + HALO
# scan-chunk output widths (sum = TLOC). Small chunks FIRST so ACT's
# Ln/Exp chain starts as early as possible (it is the pacing chain for rr:
# per-chunk Ln+Exp cost ~2*(w+222)*0.83ns+220 exceeds the scan's ~2.08*w),
# and a smallish chunk LAST so the chain's tail past the scan is short.
CHUNKS = (128, 384, 512, 768, 256)
F32 = mybir.dt.float32
BF16 = mybir.dt.bfloat16
NPBF16 = ml_dtypes.bfloat16

_nc_cache: dict = {}


def _preload_act_table(nc):
    """Pin the act table that holds BOTH Ln and Exp, so the Ln->Exp switch
    needs no mid-kernel ACT_TABLE_LOAD (1.28us each). The manual load has no
    deps, so it lands at the head of the ACT queue, overlapped with the
    input-DMA wait. insert_act_table_loads() sees the table as resident."""
    from concourse.hw_specs import get_activation_tables

    AF = mybir.ActivationFunctionType
    tabs = get_activation_tables(nc.m.arch)
    set_id = None
    for i, funcs in enumerate(tabs.values()):
        if AF.Ln in funcs and AF.Exp in funcs:
            set_id = i
            break
    if set_id is None:
        return  # fall back to automatic (greedy) table loads
    inst = mybir.InstLoadActFuncSet(
        name=nc.get_next_instruction_name(), ins=[], outs=[], act_func_set_id=set_id
    )
    inst.engine = mybir.EngineType.Activation
    nc.add_instruction(inst)


def build_nc(
    alpha: float,
    r: float,
    delta: float,
    s: float,
    eps: float,
    nbatch: int = B,
    tloc: int = TLOC,
    halo: int = HALO,
    reps: int = 1,
) -> bass.Bass:
    r_abs = abs(r)
    C = float(np.float32(delta) ** np.float32(r_abs))  # delta ** |r|
    AF = mybir.ActivationFunctionType
    OP = mybir.AluOpType
    th = tloc + halo

    # Bacc (not raw Bass): its compile() lowers multi-sem waits into separate
    # sequencer instructions; the DMA/scan ISA structs hold only one wait.
    nc = bacc.Bacc("TRN2", target_bir_lowering=False, debug=False, num_devices=NCORES)

    # xs/out are packed [F, batch*tloc] (host transposes): a [F, w] SBUF
    # tile then maps to one dma_start with a single large contiguous
    # descriptor per partition row (fewer triggers, fewer descriptors).
    xs = nc.dram_tensor("xs", [F, (nbatch - 1) * tloc], BF16, kind="ExternalInput").ap()
    x0h = nc.dram_tensor("x0h", [F, th], BF16, kind="ExternalInput").ap()
    out = nc.dram_tensor("out", [F, nbatch * tloc], BF16, kind="ExternalOutput").ap()

    with ExitStack() as ctx:
        tc = ctx.enter_context(tile.TileContext(nc))
        # singles (bufs=1 rings) live in scanp. Usable SBUF is
        # ~163KB/partition here; this layout totals ~153KB.
        scanp = ctx.enter_context(tc.tile_pool(name="scan", bufs=1))
        epool = ctx.enter_context(tc.tile_pool(name="e", bufs=2))
        espool = ctx.enter_context(tc.tile_pool(name="es", bufs=3))
        upool = ctx.enter_context(tc.tile_pool(name="u", bufs=2))
        opool = ctx.enter_context(tc.tile_pool(name="o", bufs=2))

        for _rep in range(reps):
            _build_body(nc, tc, scanp, epool, espool, upool, opool,
                        xs, x0h, out, nbatch, tloc, halo, th,
                        s, eps, alpha, delta, r_abs, C, AF, OP, _rep)
    nc.compile()
    return nc


def _build_body(nc, tc, scanp, epool, espool, upool, opool,
                xs, x0h, out, nbatch, tloc, halo, th,
                s, eps, alpha, delta, r_abs, C, AF, OP, rep):
    _preload_act_table(nc)

    # chunk boundaries in output-column space
    edges = [0]
    for w in CHUNKS:
        edges.append(edges[-1] + w)
    assert edges[-1] == tloc

    # ---- input DMAs: x0h column-chunks first, then batches in use order ----
    x0t = scanp.tile([F, th], BF16, tag="x0t")
    for ci in range(len(CHUNKS)):
        lo = 0 if ci == 0 else halo + edges[ci]
        hi = halo + edges[ci + 1]
        nc.sync.dma_start(x0t[:, lo:hi], x0h[:, lo:hi])
    # batch inputs: one DMA per batch. Grouped multi-batch DMAs were tried
    # and stall the muls: a group's completion sem fires only when its LAST
    # row lands, so per-batch granularity feeds the pipeline ~2us earlier.
    xtiles = {}
    xt = scanp.tile([F, (nbatch - 1) * tloc], BF16, tag="xall")
    for b in range(1, nbatch):
        base = (b - 1) * tloc
        nc.sync.dma_start(xt[:, base : base + tloc], xs[:, base : base + tloc])
        xtiles[b] = xt[:, base : base + tloc]

    # per-kernel const-bias tiles (activation() requires non-Copy biases
    # as APs); tile-pool deps replace the global all_engine_barrier the
    # shared const-AP registry would need.
    eps_t = scanp.tile([F, 1], F32, tag="epsc")
    nc.vector.memset(eps_t[:], float(eps))
    delta_t = scanp.tile([F, 1], F32, tag="deltac")
    nc.vector.memset(delta_t[:], float(delta))
    # single decay column broadcast along the free dim (step-0 AP)
    dcol = scanp.tile([F, 1], BF16, tag="dcol")
    nc.vector.memset(dcol[:], 1.0 - s)

    # ---- Phase A: chunked EMA scan, with Ln+Exp trailing per chunk ----
    probe = scanp.tile([F, len(CHUNKS)], BF16, tag="probe")
    m = scanp.tile([F, th], F32, tag="m")
    rr = scanp.tile([F, tloc], BF16, tag="rr")
    # scan on raw x: state = (1-s)*state + x  => true EMA m = s*state.
    # The s factor is folded into the Ln activation's input scale below.
    # Chunks are carry-chained via initial=prev m[:, -1:], so each chunk's
    # Ln+Exp overlap the next chunk's scan; Ln and Exp share the preloaded
    # table so there is no ACT_TABLE_LOAD between them.
    for ci in range(len(CHUNKS)):
        c0, c1 = edges[ci], edges[ci + 1]
        lo = 0 if ci == 0 else halo + c0  # chunk 0 includes the halo
        hi = halo + c1
        # 1-col probe copy on the vector engine carries this chunk's DMA
        # completion wait (the TensorTensorScan ISA struct has no wait slots)
        nc.vector.tensor_copy(probe[:, ci : ci + 1], x0t[:, hi - 1 : hi])
        dbc, _ = bass.broadcast_tensor_aps(dcol[:], x0t[:, lo:hi])
        init = 0.0 if ci == 0 else m[:, lo - 1 : lo]
        nc.vector.tensor_tensor_scan(
            m[:, lo:hi], dbc, x0t[:, lo:hi], init, OP.mult, OP.add
        )
        # R = exp(-alpha*ln(s*m' + eps)) == (M+eps)^-alpha  (M+eps > 0).
        # lnm is a 1-buf ring: Ln_{c+1}'s write waits on Exp_c's read, which
        # pins the scheduler to strict Ln/Exp alternation (a freely-ordered
        # scheduler batched the Lns and delayed early rr chunks by >1us).
        w = c1 - c0
        lnm = scanp.tile([F, max(CHUNKS)], F32, tag="lnm")
        nc.scalar.activation(
            lnm[:, :w], m[:, halo + c0 : halo + c1], AF.Ln,
            bias=eps_t[:], scale=float(s),
        )
        nc.scalar.activation(
            rr[:, c0:c1], lnm[:, :w], AF.Exp, scale=-float(alpha)
        )

    # ---- Phase B: per-batch elementwise PCEN ----
    # groups: (batch-list, split-sub) -- batches in one group share a Sqrt
    # and a sub via a [F, 4096] pair tile. The 16 queues drain outputs at
    # only ~360GB/s aggregate while the subs can produce faster, so order
    # output sizes DESCENDING: big pair outputs early (drain while compute
    # continues), singles late, and the last batch's sub split in halves so
    # the final queued bytes after the last sub are small.
    groups = [([0], False), ([1, 2], False), ([3, 4], False), ([5], False),
              ([6], False), ([7], True)]
    assert sorted(b for g, _ in groups for b in g) == list(range(nbatch))

    def xsrc(b):
        return x0t[:, halo:] if b == 0 else xtiles[b][:]

    def emit_mul(e, off, b, split_last):
        # E = x / M^alpha; DVE bf16 2x mode. The first mul is split at the
        # last chunk edge so it can start before the final Exp chunk lands.
        if split_last:
            cut = edges[len(CHUNKS) - 1]
            nc.vector.tensor_mul(e[:, off : off + cut], xsrc(b)[:, :cut], rr[:, :cut])
            nc.vector.tensor_mul(
                e[:, off + cut : off + tloc], xsrc(b)[:, cut:], rr[:, cut:]
            )
        else:
            nc.vector.tensor_mul(e[:, off : off + tloc], xsrc(b), rr[:])

    def emit_sqrt(u, e):
        if r_abs == 0.5:
            nc.scalar.activation(u[:], e[:], AF.Sqrt, bias=delta_t[:])
        else:
            nc.scalar.activation(u[:], e[:], AF.Ln, bias=delta_t[:])
            nc.scalar.activation(u[:], u[:], AF.Exp, scale=float(r_abs))

    def emit_sub(g, u, batches, split):
        # out = U - delta^r; must read fp32 U (cancellation near U ~ C)
        w = len(batches) * tloc
        if split:
            h = w // 2
            o1 = scanp.tile([F, h], BF16, tag="osa")
            nc.vector.tensor_scalar_add(o1[:], u[:, :h], -C)
            o2 = scanp.tile([F, w - h], BF16, tag="osb")
            nc.vector.tensor_scalar_add(o2[:], u[:, h:], -C)
            otiles = [(o1, 0, h), (o2, h, w)]
        elif len(batches) == 1:
            o = opool.tile([F, w], BF16, tag="os")
            nc.vector.tensor_scalar_add(o[:], u[:], -C)
            otiles = [(o, 0, w)]
        else:
            o = opool.tile([F, w], BF16, tag="op")
            nc.vector.tensor_scalar_add(o[:], u[:], -C)
            otiles = [(o, 0, w)]
        # batches in a group are consecutive, so each o tile is one
        # contiguous span of the packed out tensor: a single dma_start.
        base = batches[0] * tloc
        for o, olo, ohi in otiles:
            nc.sync.dma_start(out[:, base + olo : base + ohi], o[:])

    # software-pipelined emission: DVE runs in order, so each group's sub is
    # emitted ~1.5 groups after its mul, keeping DVE fed while ACT's Sqrt
    # catches up.
    etiles = {}
    utiles = {}
    pend = []  # groups whose sub is not yet emitted
    for gi, (batches, split) in enumerate(groups):
        w = len(batches) * tloc
        if len(batches) == 2:
            e = epool.tile([F, w], BF16, tag="ep")
            u = upool.tile([F, w], F32, tag="up")
        else:
            e = espool.tile([F, w], BF16, tag="es")
            u = upool.tile([F, w], F32, tag="us")
        for k, b in enumerate(batches):
            emit_mul(e, k * tloc, b, split_last=(gi == 0))
        emit_sqrt(u, e)
        etiles[gi], utiles[gi] = e, u
        pend.append(gi)
        # emit the oldest pending sub once we're 2 groups ahead
        if len(pend) > 2:
            g0 = pend.pop(0)
            emit_sub(g0, utiles[g0], groups[g0][0], groups[g0][1])
    for g0 in pend:
        if groups[g0][1]:
            # pin the split final subs (and their out-DMAs) to the very end
            # of the schedule: the scheduler's internal sim otherwise hoists
            # them before the last pair sub, which makes a 1MB pair output
            # the final DMA and stretches the drain by ~2us.
            with tc.tile_wait_until(0.1):
                emit_sub(g0, utiles[g0], groups[g0][0], groups[g0][1])
        else:
            emit_sub(g0, utiles[g0], groups[g0][0], groups[g0][1])


def _get_nc(alpha, r, delta, s, eps):
    key = (alpha, r, delta, s, eps)
    if key not in _nc_cache:
        _nc_cache[key] = build_nc(alpha, r, delta, s, eps)
    return _nc_cache[key]


def make_in_maps(data: np.ndarray) -> list[dict]:
    """Shard the full [B,F,T] input into per-core input maps (T-sharding).

    xs is packed [F, (B-1)*TLOC] with batch-major columns, so each on-device
    batch group loads as one DMA with a single contiguous descriptor per
    partition row."""
    data16 = data.astype(NPBF16)
    in_maps = []
    for c in range(NCORES):
        t0, t1 = c * TLOC, (c + 1) * TLOC
        xs_c = np.ascontiguousarray(
            data16[1:, :, t0:t1].transpose(1, 0, 2).reshape(F, (B - 1) * TLOC)
        )
        x0h_c = np.zeros((F, TH), NPBF16)
        lo = max(0, t0 - HALO)
        x0h_c[:, HALO - (t0 - lo) :] = data16[0, :, lo:t1]
        in_maps.append({"xs": xs_c, "x0h": x0h_c})
    return in_maps


def kernel(data, alpha, r, delta, s, eps, _trace=False):
    data = np.ascontiguousarray(np.asarray(data, dtype=np.float32))
    assert data.shape == (B, F, T), data.shape
    a, rv, dv, sv, ev = (float(np.asarray(v)) for v in (alpha, r, delta, s, eps))
    nc = _get_nc(a, rv, dv, sv, ev)
    in_maps = make_in_maps(data)
    res = run_bass_kernel_spmd(nc, in_maps, list(range(NCORES)), trace=_trace)
    outp = np.empty((B, F, T), np.float32)
    for c in range(NCORES):
        # out is packed [F, B*TLOC] batch-major; unpack to [B, F, TLOC]
        oc = np.asarray(res.results[c]["out"]).reshape(F, B, TLOC)
        outp[:, :, c * TLOC : (c + 1) * TLOC] = oc.transpose(1, 0, 2).astype(
            np.float32
        )
    if _trace:
        return outp, res
    return outp


# revision 32
# speedup vs baseline: 1.0125x; 1.0125x over previous
"""PCEN kernel for Trainium2, sharded over the time axis across 8 NeuronCores.

Strategy (v2):
  - data is [B=8, F=128, T=16384] fp32. Each core owns a T-slice of 2048 cols
    (all batches). Inputs are downcast to bf16 on the host and outputs are
    shipped back as bf16 (rel tolerance is 2e-2; bf16 rounding is ~2e-3 max),
    halving DMA traffic.
  - The EMA smoother M uses only batch 0. Each core computes M for its own
    slice independently via a 32-col halo: contributions older than 32 steps
    are attenuated by (1-s)^32 ~ 2e-10. The scan is 4 chained
    TensorTensorScan chunks (fp32 state), with x0h DMA'd in 4 matching
    column-chunks so scan chunk c starts as soon as its slice lands
    (per-chunk 1-col probe copies carry the DMA waits; the scan ISA struct
    has no wait slots).
  - R = 1/M^alpha = exp(-alpha*ln(M+eps)) via per-chunk Ln+Exp on ACT. A
    manual InstLoadActFuncSet pins the natural_log_exp_and_others table at
    program start (overlapped with the DMA wait), so Ln->Exp needs NO table
    reload and the chunks pipeline behind the scan; only the Sqrt table
    load (1.28us) remains, right after the last Exp.
  - Chunk widths [672|640|512|256] (first includes the 32-col halo): the
    last chunk is small so the trailing Ln+Exp adds only ~0.8us after the
    scan instead of a full-width 2us Ln + 2us Exp.
  - Per batch b: E = x*R (DVE bf16 2x mode), U = sqrt(E + delta) (ACT fp32),
    out = U - delta^r (DVE tensor_scalar, fp32 in / bf16 out). The subtract
    must read fp32 U: near-zero outputs cancel (U ~ sqrt(delta)), so a bf16
    U would blow up the relative error. GPSIMD is excluded from the hot
    path: its tensor_scalar runs at ~9 G elem/s vs DVE's ~240 G elem/s.
  - Batches 1..6 are processed as pairs in [F, 4096] tiles: one Sqrt and one
    sub per pair halves the per-instruction overhead (ACT pays 222
    init-cycles per instruction). The last batch's sub is split into two
    [F, 1024] halves so the final output DMA is small (shorter drain).
  - All input DMAs are queued upfront on the sync (SP) HWDGE ring, x0h
    chunks first then batches in compute order: the per-queue rings drain
    FIFO, so x0h still lands first without a serializing gate DMA.
"""

import sys

if "/opt/trn_rl_repo" not in sys.path:
    sys.path.insert(0, "/opt/trn_rl_repo")

from contextlib import ExitStack

import ml_dtypes
import numpy as np

import concourse.bass as bass
import concourse.mybir as mybir
import concourse.tile as tile
from concourse import bacc
from concourse.bass_utils import run_bass_kernel_spmd

B, F, T = 8, 128, 16384
NCORES = 8
TLOC = T // NCORES  # 2048 cols per core
HALO = 32  # scan warmup; (1-s)^32 ~ 2e-10 << 2e-2 tolerance
TH = TLOC + HALO
# scan-chunk output widths (sum = TLOC). Small chunks FIRST so ACT's
# Ln/Exp chain starts as early as possible (it is the pacing chain for rr:
# per-chunk Ln+Exp cost ~2*(w+222)*0.83ns+220 exceeds the scan's ~2.08*w),
# and a smallish chunk LAST so the chain's tail past the scan is short.
CHUNKS = (128, 256, 512, 640, 512)
F32 = mybir.dt.float32
BF16 = mybir.dt.bfloat16
NPBF16 = ml_dtypes.bfloat16

_nc_cache: dict = {}


def _preload_act_table(nc):
    """Pin the act table that holds BOTH Ln and Exp, so the Ln->Exp switch
    needs no mid-kernel ACT_TABLE_LOAD (1.28us each). The manual load has no
    deps, so it lands at the head of the ACT queue, overlapped with the
    input-DMA wait. insert_act_table_loads() sees the table as resident."""
    from concourse.hw_specs import get_activation_tables

    AF = mybir.ActivationFunctionType
    tabs = get_activation_tables(nc.m.arch)
    set_id = None
    for i, funcs in enumerate(tabs.values()):
        if AF.Ln in funcs and AF.Exp in funcs:
            set_id = i
            break
    if set_id is None:
        return  # fall back to automatic (greedy) table loads
    inst = mybir.InstLoadActFuncSet(
        name=nc.get_next_instruction_name(), ins=[], outs=[], act_func_set_id=set_id
    )
    inst.engine = mybir.EngineType.Activation
    nc.add_instruction(inst)


def build_nc(
    alpha: float,
    r: float,
    delta: float,
    s: float,
    eps: float,
    nbatch: int = B,
    tloc: int = TLOC,
    halo: int = HALO,
    reps: int = 1,
) -> bass.Bass:
    r_abs = abs(r)
    C = float(np.float32(delta) ** np.float32(r_abs))  # delta ** |r|
    AF = mybir.ActivationFunctionType
    OP = mybir.AluOpType
    th = tloc + halo

    # Bacc (not raw Bass): its compile() lowers multi-sem waits into separate
    # sequencer instructions; the DMA/scan ISA structs hold only one wait.
    nc = bacc.Bacc("TRN2", target_bir_lowering=False, debug=False, num_devices=NCORES)

    # xs/out are packed [F, batch*tloc] (host transposes): a [F, w] SBUF
    # tile then maps to one dma_start with a single large contiguous
    # descriptor per partition row (fewer triggers, fewer descriptors).
    xs = nc.dram_tensor("xs", [F, (nbatch - 1) * tloc], BF16, kind="ExternalInput").ap()
    x0h = nc.dram_tensor("x0h", [F, th], BF16, kind="ExternalInput").ap()
    out = nc.dram_tensor("out", [F, nbatch * tloc], BF16, kind="ExternalOutput").ap()

    with ExitStack() as ctx:
        tc = ctx.enter_context(tile.TileContext(nc))
        # singles (bufs=1 rings) live in scanp. Usable SBUF is
        # ~163KB/partition here; this layout totals ~153KB.
        scanp = ctx.enter_context(tc.tile_pool(name="scan", bufs=1))
        epool = ctx.enter_context(tc.tile_pool(name="e", bufs=2))
        espool = ctx.enter_context(tc.tile_pool(name="es", bufs=3))
        upool = ctx.enter_context(tc.tile_pool(name="u", bufs=2))
        # singles' u ring must be 3 deep: with 2, sqrt_b7 WAR-waits on
        # sub_b5, stalling the ACT sqrt chain by ~0.7us
        uspool = ctx.enter_context(tc.tile_pool(name="us", bufs=3))
        opool = ctx.enter_context(tc.tile_pool(name="o", bufs=2))

        for _rep in range(reps):
            _build_body(nc, tc, scanp, epool, espool, upool, uspool, opool,
                        xs, x0h, out, nbatch, tloc, halo, th,
                        s, eps, alpha, delta, r_abs, C, AF, OP, _rep)
    nc.compile()
    return nc


def _build_body(nc, tc, scanp, epool, espool, upool, uspool, opool,
                xs, x0h, out, nbatch, tloc, halo, th,
                s, eps, alpha, delta, r_abs, C, AF, OP, rep):
    _preload_act_table(nc)

    # chunk boundaries in output-column space
    edges = [0]
    for w in CHUNKS:
        edges.append(edges[-1] + w)
    assert edges[-1] == tloc

    # ---- input DMAs: x0h column-chunks first, then batches in use order ----
    x0t = scanp.tile([F, th], BF16, tag="x0t")
    for ci in range(len(CHUNKS)):
        lo = 0 if ci == 0 else halo + edges[ci]
        hi = halo + edges[ci + 1]
        nc.sync.dma_start(x0t[:, lo:hi], x0h[:, lo:hi])
    # batch inputs: one DMA per batch. Grouped multi-batch DMAs were tried
    # and stall the muls: a group's completion sem fires only when its LAST
    # row lands, so per-batch granularity feeds the pipeline ~2us earlier.
    xtiles = {}
    xt = scanp.tile([F, (nbatch - 1) * tloc], BF16, tag="xall")
    for b in range(1, nbatch):
        base = (b - 1) * tloc
        nc.sync.dma_start(xt[:, base : base + tloc], xs[:, base : base + tloc])
        xtiles[b] = xt[:, base : base + tloc]

    # per-kernel const-bias tiles (activation() requires non-Copy biases
    # as APs); tile-pool deps replace the global all_engine_barrier the
    # shared const-AP registry would need.
    eps_t = scanp.tile([F, 1], F32, tag="epsc")
    nc.vector.memset(eps_t[:], float(eps))
    delta_t = scanp.tile([F, 1], F32, tag="deltac")
    nc.vector.memset(delta_t[:], float(delta))
    # single decay column broadcast along the free dim (step-0 AP)
    dcol = scanp.tile([F, 1], BF16, tag="dcol")
    nc.vector.memset(dcol[:], 1.0 - s)

    # ---- Phase A: chunked EMA scan, with Ln+Exp trailing per chunk ----
    probe = scanp.tile([F, len(CHUNKS)], BF16, tag="probe")
    m = scanp.tile([F, th], F32, tag="m")
    rr = scanp.tile([F, tloc], BF16, tag="rr")
    # scan on raw x: state = (1-s)*state + x  => true EMA m = s*state.
    # The s factor is folded into the Ln activation's input scale below.
    # Chunks are carry-chained via initial=prev m[:, -1:], so each chunk's
    # Ln+Exp overlap the next chunk's scan; Ln and Exp share the preloaded
    # table so there is no ACT_TABLE_LOAD between them.
    for ci in range(len(CHUNKS)):
        c0, c1 = edges[ci], edges[ci + 1]
        lo = 0 if ci == 0 else halo + c0  # chunk 0 includes the halo
        hi = halo + c1
        # 1-col probe copy on the vector engine carries this chunk's DMA
        # completion wait (the TensorTensorScan ISA struct has no wait slots)
        nc.vector.tensor_copy(probe[:, ci : ci + 1], x0t[:, hi - 1 : hi])
        dbc, _ = bass.broadcast_tensor_aps(dcol[:], x0t[:, lo:hi])
        init = 0.0 if ci == 0 else m[:, lo - 1 : lo]
        nc.vector.tensor_tensor_scan(
            m[:, lo:hi], dbc, x0t[:, lo:hi], init, OP.mult, OP.add
        )
        # R = exp(-alpha*ln(s*m' + eps)) == (M+eps)^-alpha  (M+eps > 0).
        # lnm is a 1-buf ring: Ln_{c+1}'s write waits on Exp_c's read, which
        # pins the scheduler to strict Ln/Exp alternation (a freely-ordered
        # scheduler batched the Lns and delayed early rr chunks by >1us).
        w = c1 - c0
        lnm = scanp.tile([F, max(CHUNKS)], F32, tag="lnm")
        nc.scalar.activation(
            lnm[:, :w], m[:, halo + c0 : halo + c1], AF.Ln,
            bias=eps_t[:], scale=float(s),
        )
        nc.scalar.activation(
            rr[:, c0:c1], lnm[:, :w], AF.Exp, scale=-float(alpha)
        )

    # ---- Phase B: per-batch elementwise PCEN ----
    # groups: (batch-list, split-sub) -- batches in one group share a Sqrt
    # and a sub via a [F, 4096] pair tile. The 16 queues drain outputs at
    # only ~360GB/s aggregate while the subs can produce faster, so order
    # output sizes DESCENDING: big pair outputs early (drain while compute
    # continues), singles late, and the last batch's sub split in halves so
    # the final queued bytes after the last sub are small.
    groups = [([0], False), ([1, 2], False), ([3, 4], False), ([5], False),
              ([6], False), ([7], True)]
    assert sorted(b for g, _ in groups for b in g) == list(range(nbatch))

    def xsrc(b):
        return x0t[:, halo:] if b == 0 else xtiles[b][:]

    def emit_mul(e, off, b, split_last):
        # E = x / M^alpha; DVE bf16 2x mode. The first mul is split at the
        # last chunk edge so it can start before the final Exp chunk lands.
        if split_last:
            cut = edges[len(CHUNKS) - 1]
            nc.vector.tensor_mul(e[:, off : off + cut], xsrc(b)[:, :cut], rr[:, :cut])
            nc.vector.tensor_mul(
                e[:, off + cut : off + tloc], xsrc(b)[:, cut:], rr[:, cut:]
            )
        else:
            nc.vector.tensor_mul(e[:, off : off + tloc], xsrc(b), rr[:])

    def emit_sqrt(u, e):
        if r_abs == 0.5:
            nc.scalar.activation(u[:], e[:], AF.Sqrt, bias=delta_t[:])
        else:
            nc.scalar.activation(u[:], e[:], AF.Ln, bias=delta_t[:])
            nc.scalar.activation(u[:], u[:], AF.Exp, scale=float(r_abs))

    def emit_sub(g, u, batches, split):
        # out = U - delta^r; must read fp32 U (cancellation near U ~ C)
        w = len(batches) * tloc
        if split:
            h = w // 2
            o1 = scanp.tile([F, h], BF16, tag="osa")
            nc.vector.tensor_scalar_add(o1[:], u[:, :h], -C)
            o2 = scanp.tile([F, w - h], BF16, tag="osb")
            nc.vector.tensor_scalar_add(o2[:], u[:, h:], -C)
            otiles = [(o1, 0, h), (o2, h, w)]
        elif len(batches) == 1:
            o = opool.tile([F, w], BF16, tag="os")
            nc.vector.tensor_scalar_add(o[:], u[:], -C)
            otiles = [(o, 0, w)]
        else:
            o = opool.tile([F, w], BF16, tag="op")
            nc.vector.tensor_scalar_add(o[:], u[:], -C)
            otiles = [(o, 0, w)]
        # batches in a group are consecutive, so each o tile is one
        # contiguous span of the packed out tensor: a single dma_start.
        base = batches[0] * tloc
        for o, olo, ohi in otiles:
            nc.sync.dma_start(out[:, base + olo : base + ohi], o[:])

    # software-pipelined emission: DVE runs in order, so each group's sub is
    # emitted ~1.5 groups after its mul, keeping DVE fed while ACT's Sqrt
    # catches up.
    etiles = {}
    utiles = {}
    pend = []  # groups whose sub is not yet emitted
    for gi, (batches, split) in enumerate(groups):
        w = len(batches) * tloc
        if len(batches) == 2:
            e = epool.tile([F, w], BF16, tag="ep")
            u = upool.tile([F, w], F32, tag="up")
        else:
            e = espool.tile([F, w], BF16, tag="es")
            u = uspool.tile([F, w], F32, tag="us")
        for k, b in enumerate(batches):
            emit_mul(e, k * tloc, b, split_last=(gi == 0))
        emit_sqrt(u, e)
        etiles[gi], utiles[gi] = e, u
        pend.append(gi)
        # emit the oldest pending sub once we're 2 groups ahead
        if len(pend) > 2:
            g0 = pend.pop(0)
            emit_sub(g0, utiles[g0], groups[g0][0], groups[g0][1])
    for g0 in pend:
        if groups[g0][1]:
            # pin the split final subs (and their out-DMAs) to the very end
            # of the schedule: the scheduler's internal sim otherwise hoists
            # them before the last pair sub, which makes a 1MB pair output
            # the final DMA and stretches the drain by ~2us.
            with tc.tile_wait_until(0.1):
                emit_sub(g0, utiles[g0], groups[g0][0], groups[g0][1])
        else:
            emit_sub(g0, utiles[g0], groups[g0][0], groups[g0][1])


def _get_nc(alpha, r, delta, s, eps):
    key = (alpha, r, delta, s, eps)
    if key not in _nc_cache:
        _nc_cache[key] = build_nc(alpha, r, delta, s, eps)
    return _nc_cache[key]


def make_in_maps(data: np.ndarray) -> list[dict]:
    """Shard the full [B,F,T] input into per-core input maps (T-sharding).

    xs is packed [F, (B-1)*TLOC] with batch-major columns, so each on-device
    batch group loads as one DMA with a single contiguous descriptor per
    partition row."""
    data16 = data.astype(NPBF16)
    in_maps = []
    for c in range(NCORES):
        t0, t1 = c * TLOC, (c + 1) * TLOC
        xs_c = np.ascontiguousarray(
            data16[1:, :, t0:t1].transpose(1, 0, 2).reshape(F, (B - 1) * TLOC)
        )
        x0h_c = np.zeros((F, TH), NPBF16)
        lo = max(0, t0 - HALO)
        x0h_c[:, HALO - (t0 - lo) :] = data16[0, :, lo:t1]
        in_maps.append({"xs": xs_c, "x0h": x0h_c})
    return in_maps


def kernel(data, alpha, r, delta, s, eps, _trace=False):
    data = np.ascontiguousarray(np.asarray(data, dtype=np.float32))
    assert data.shape == (B, F, T), data.shape
    a, rv, dv, sv, ev = (float(np.asarray(v)) for v in (alpha, r, delta, s, eps))
    nc = _get_nc(a, rv, dv, sv, ev)
    in_maps = make_in_maps(data)
    res = run_bass_kernel_spmd(nc, in_maps, list(range(NCORES)), trace=_trace)
    outp = np.empty((B, F, T), np.float32)
    for c in range(NCORES):
        # out is packed [F, B*TLOC] batch-major; unpack to [B, F, TLOC]
        oc = np.asarray(res.results[c]["out"]).reshape(F, B, TLOC)
        outp[:, :, c * TLOC : (c + 1) * TLOC] = oc.transpose(1, 0, 2).astype(
            np.float32
        )
    if _trace:
        return outp, res
    return outp
